# revision 39
# baseline (speedup 1.0000x reference)
"""GATv2 layer on 8 Trainium2 NeuronCores (Bass/Tile) — v4.

Sharding: dst nodes are load-balanced (serpentine by degree + swap repair)
into 8*49 windows of <=128 nodes so every window holds ~2041 edges and the
per-window tile count K is minimal (16).  Device d owns 49 windows.  The host
ships, per edge slot, feature-major fp16 gathered src|dst node features plus
a precomputed fp16 one-hot (edge -> window node) used to express the segment
softmax sums as PSUM matmuls.  fp16 keeps the 2 KB-per-partition DMA lines
that sustain ~420 GB/s and beats fp8 on LDWEIGHTS speed; fp8 saved no time.

Per group of 512 edges (4 tiles x 128):
  PE    z0 [hf,512] = WsT.T @ seT + WdT.T @ deT       (2 matmuls)
  ACT   ay = |z0|      (Abs shares the resident exp table; Lrelu does NOT --
                        it costs a 1.3us ACT_TABLE_LOAD per group.  score =
                        0.6*attn.z_src + 0.4*attn*|z|; the 0.6*attn.z_dst
                        term is constant per dst and cancels in softmax.)
  PE    scE [128,t,4] = seT_t.T @ wu  (start)
                      + ay_t.T @ (0.4*attn)  (stop; identical out region --
                        PSUM accumulation with a sub-region second write
                        corrupts the data, see v3 post-mortem)
  PE    fs [128,t,HF] = seT_t.T @ Ws
  ACT   msgex[:,:,HF:] = Exp(scE)                     (softmax max-shift
                                                       skipped: scores O(0.3))
  DVE   msgex[:,:,:HF] = fs * ex                      (broadcast over f)
  PE    acc [n,HF+4]  += oh_t.T @ msgex_t             (segment sums)
Window tail: out = acc[:,:HF]/max(acc[:,HF:],eps) + res (residual projection
on PE into its own PSUM bank, added by DVE).

Software pipeline (per iteration i): DMA for group i+4 (the scheduler only
releases a DMA near its emission point, so prefetch must be explicit in
program order), z0+ABS for group i, fs/score/exp/msgex for group i-2, acc
for group i-4 — every cross-engine dependency is >=1 iteration old.  ay and
msgex live in their own tile pools so pool write-barriers never couple the
ABS (which gates z0 PSUM reuse) to the DVE window-tail chain.  Out-DMAs ride
the gpsimd queue whose oh prefetch absorbs their ob2-wait head-of-line block.
"""
import sys
import numpy as np

sys.path.insert(0, "/opt/trn_rl_repo")

import concourse.bass as bass  # noqa: E402
import concourse.bacc as bacc  # noqa: E402
import concourse.tile as tile  # noqa: E402
from concourse import mybir  # noqa: E402
from concourse.bass_utils import run_bass_kernel_spmd  # noqa: E402

IN_FEATS = 128
N_HEADS = 4
OUT_FEATS = 32
HF = N_HEADS * OUT_FEATS  # 128
SLOPE = 0.2
P = 128
GRP = 4                              # tiles per group (512 edges)
GPP = 28                             # sede groups per dram part (even: DMA
                                     # fetches 2-group pairs; 28*2048<=64K)
OSPLIT = 4                           # dram parts for oh

N_NODES = 50000
M = 8
WIN = 49                             # windows per device (49*128 = 6272 rows)
OUT_ROWS = WIN * P

f16 = mybir.dt.float16
f32 = mybir.dt.float32

_prog_cache = {}


def _build_program(K: int, n_groups: int):
    nc = bacc.Bacc("TRN2", debug=False, num_devices=M)
    n_tiles_pad = n_groups * GRP

    assert n_groups % 2 == 0
    nsplit = (n_groups + GPP - 1) // GPP
    sede_p = [nc.dram_tensor(f"sede{i}", [P, 2 * GRP * P * min(GPP, n_groups - i * GPP)],
                             f16, kind="ExternalInput") for i in range(nsplit)]
    opp = (n_groups + OSPLIT - 1) // OSPLIT
    oh_p = [nc.dram_tensor(f"oh{i}", [P, GRP * P * min(opp, n_groups - i * opp)],
                           f16, kind="ExternalInput") for i in range(OSPLIT)]
    fwinT = nc.dram_tensor("fwinT", [P, OUT_ROWS], f16, kind="ExternalInput")
    ws_d = nc.dram_tensor("ws16", [P, HF], f16, kind="ExternalInput")
    wd_d = nc.dram_tensor("wd16", [P, HF], f16, kind="ExternalInput")
    wu_d = nc.dram_tensor("wu16", [P, N_HEADS], f16, kind="ExternalInput")
    wr_d = nc.dram_tensor("wr16", [P, HF], f16, kind="ExternalInput")
    asel_d = nc.dram_tensor("asel", [HF, N_HEADS], f16, kind="ExternalInput")
    out_d = nc.dram_tensor("out_d", [OUT_ROWS, HF], f32, kind="ExternalOutput")

    mult = mybir.AluOpType.mult
    addop = mybir.AluOpType.add
    maxop = mybir.AluOpType.max

    with tile.TileContext(nc) as tc:
        with tc.tile_pool(name="const", bufs=1) as cpool:
            ws = cpool.tile([P, HF], f16, tag="ws")
            wd = cpool.tile([P, HF], f16, tag="wd")
            wu = cpool.tile([P, N_HEADS], f16, tag="wu")
            wr = cpool.tile([P, HF], f16, tag="wr")
            asel = cpool.tile([HF, N_HEADS], f16, tag="asel")
            fwin = cpool.tile([P, OUT_ROWS], f16, tag="fwin")
            nc.sync.dma_start(ws[:], ws_d[:])
            nc.sync.dma_start(wd[:], wd_d[:])
            nc.sync.dma_start(wu[:], wu_d[:])
            nc.sync.dma_start(wr[:], wr_d[:])
            nc.sync.dma_start(asel[:], asel_d[:])
            nc.sync.dma_start(fwin[:], fwinT[:])

            with tc.tile_pool(name="pb", bufs=5) as pb, \
                 tc.tile_pool(name="pay", bufs=4) as pay, \
                 tc.tile_pool(name="pmsg", bufs=4) as pmsg, \
                 tc.tile_pool(name="pw", bufs=4) as pw, \
                 tc.tile_pool(name="ps_z", bufs=2, space="PSUM") as ps_z, \
                 tc.tile_pool(name="ps_fs", bufs=2, space="PSUM") as ps_fs, \
                 tc.tile_pool(name="ps_s", bufs=1, space="PSUM") as ps_s, \
                 tc.tile_pool(name="ps_r", bufs=1, space="PSUM") as ps_r, \
                 tc.tile_pool(name="ps_acc", bufs=2, space="PSUM") as ps_acc:

                st = {"acc": None}

                def emit_acc_block(g, msgex_t, oh_t):
                    """acc matmuls + window tails for group g (2 iters old)."""
                    for t in range(GRP):
                        tau = g * GRP + t
                        w = min(tau // K, WIN - 1)
                        k = tau - w * K
                        last_k = (K - 1) if w < WIN - 1 else (n_tiles_pad - 1 - w * K)
                        if k == 0:
                            st["acc"] = ps_acc.tile([P, HF + N_HEADS], f32,
                                                    tag="acc", name="acc")
                        acc = st["acc"]
                        nc.tensor.matmul(acc[:], lhsT=oh_t[:, t, :],
                                         rhs=msgex_t[:, t, :],
                                         start=(k == 0), stop=(k == last_k))
                        if k == last_k:
                            den = pw.tile([P, N_HEADS], f32, tag="den")
                            nc.vector.tensor_scalar(
                                out=den[:], in0=acc[:, HF:],
                                scalar1=1e-30, scalar2=None, op0=maxop)
                            rec = pw.tile([P, N_HEADS], f32, tag="rec")
                            nc.vector.reciprocal(out=rec[:], in_=den[:])
                            res_ps = ps_r.tile([P, HF], f32, tag="res",
                                               name="res_ps")
                            nc.tensor.matmul(res_ps[:],
                                             lhsT=fwin[:, w * P:(w + 1) * P],
                                             rhs=wr[:], start=True, stop=True)
                            osb = pw.tile([P, HF], f32, tag="osb")
                            nc.vector.tensor_tensor(
                                out=osb.rearrange("p (h f) -> p h f", h=N_HEADS),
                                in0=acc[:, 0:HF].rearrange(
                                    "p (h f) -> p h f", h=N_HEADS),
                                in1=rec[:][:, :, None].to_broadcast(
                                    [P, N_HEADS, OUT_FEATS]),
                                op=mult)
                            ob2 = pw.tile([P, HF], f32, tag="ob2")
                            nc.vector.tensor_tensor(
                                out=ob2[:], in0=res_ps[:], in1=osb[:],
                                op=addop)
                            nc.gpsimd.dma_start(
                                out_d[w * P:(w + 1) * P, :], ob2[:])

                hist = {}  # group -> (sd, ohb, ay, scE, fs, msgex)

                AHEAD = 4

                def emit_dma(g):
                    """DMA-in group g's tiles (emitted AHEAD iters early so
                    the transfers run well before the data is needed — the
                    scheduler only releases a DMA near its emission point in
                    program order, so bufs alone don't create prefetch)."""
                    if g % 2 == 0:
                        pi, go = divmod(g, GPP)
                        sdd = pb.tile([P, 2, 2, GRP * P], f16, tag="sede",
                                      name="sdd", bufs=5)
                        nc.sync.dma_start(
                            sdd.rearrange("p q j e -> p (q j e)"),
                            sede_p[pi][:, 2 * GRP * P * go:
                                       2 * GRP * P * (go + 2)])
                        dmad[g] = sdd[:, 0]
                        dmad[g + 1] = sdd[:, 1]
                    oi, oo = divmod(g, opp)
                    ohb = pb.tile([P, GRP, P], f16, tag="oh", bufs=9,
                                  name="ohb")
                    nc.gpsimd.dma_start(
                        ohb.rearrange("p t n -> p (t n)"),
                        oh_p[oi][:, GRP * P * oo:GRP * P * (oo + 1)])
                    dmao[g] = ohb

                dmad = {}
                dmao = {}
                for g in range(min(AHEAD, n_groups)):
                    emit_dma(g)

                for i in range(n_groups + 4):
                    if i + AHEAD < n_groups:
                        emit_dma(i + AHEAD)
                    if i < n_groups:
                        g = i
                        sd = dmad.pop(g)
                        ohb = dmao[g]

                        z0 = ps_z.tile([P, GRP * P], f32, tag="z0")
                        nc.tensor.matmul(z0[:], lhsT=ws[:], rhs=sd[:, 0, :],
                                         start=True, stop=False)
                        nc.tensor.matmul(z0[:], lhsT=wd[:], rhs=sd[:, 1, :],
                                         start=False, stop=True)
                        # ay for group i on ACT, emitted before stage-2's exp
                        # so the z0(i+2) PSUM WAR clears as early as possible
                        ay = pay.tile([P, GRP * P], f16, tag="ay")
                        nc.scalar.activation(
                            out=ay[:], in_=z0[:],
                            func=mybir.ActivationFunctionType.Abs)

                    # stage-2 for group i-2 (PE; ay two iters old so ACT can
                    # never be the blocker; wu/ay pairs adjacent with
                    # identical out regions -- required for correct PSUM
                    # accumulation)
                    if 2 <= i <= n_groups + 1:
                        sd1, _, ay1, _ = hist[i - 2]
                        fs1 = ps_fs.tile([P, GRP, HF], f32, tag="fs",
                                         name="fs1")
                        for t in range(GRP):
                            sl = slice(t * P, (t + 1) * P)
                            nc.tensor.matmul(fs1[:, t, :], lhsT=sd1[:, 0, sl],
                                             rhs=ws[:], start=True, stop=True)
                        scE1 = ps_s.tile([P, GRP, N_HEADS], f32,
                                         tag="scE", name="scE1")
                        for t in range(GRP):
                            sl = slice(t * P, (t + 1) * P)
                            nc.tensor.matmul(
                                scE1[:, t, :], lhsT=sd1[:, 0, sl],
                                rhs=wu[:], start=True, stop=False)
                            nc.tensor.matmul(
                                scE1[:, t, :], lhsT=ay1[:, sl],
                                rhs=asel[:], start=False, stop=True)
                        msgex1 = pmsg.tile([P, GRP, HF + N_HEADS], f16,
                                         tag="msgex", name="msgex1")
                        nc.scalar.activation(
                            out=msgex1[:, :, HF:], in_=scE1[:],
                            func=mybir.ActivationFunctionType.Exp)
                        nc.vector.tensor_tensor(
                            out=msgex1[:, :, 0:HF].rearrange(
                                "p t (h f) -> p t h f", h=N_HEADS),
                            in0=fs1[:].rearrange(
                                "p t (h f) -> p t h f", h=N_HEADS),
                            in1=msgex1[:, :, HF:][:, :, :, None].to_broadcast(
                                [P, GRP, N_HEADS, OUT_FEATS]),
                            op=mult)
                        hist[i - 2] = (sd1, hist[i - 2][1], ay1, msgex1)

                    if i < n_groups:
                        hist[g] = (sd, ohb, ay, None)

                    # stage-3 acc for group i-4 (msgex two iters old)
                    if i >= 4:
                        _, ohb2, _, msgex2 = hist[i - 4]
                        emit_acc_block(i - 4, msgex2, ohb2)
                        del hist[i - 4]

    nc.compile()
    return nc


def _balance(dst):
    """Assign nodes to M*WIN windows (<=128 nodes each) balancing edge load.

    Serpentine deal by descending degree, then swap-repair the worst windows.
    Returns (node_w, node_pos, K)."""
    deg = np.bincount(dst, minlength=N_NODES)
    NW = M * WIN
    order = np.argsort(-deg, kind="stable")
    node_w = np.empty(N_NODES, np.int64)
    node_pos = np.empty(N_NODES, np.int64)
    r = np.arange(N_NODES) // NW
    c = np.arange(N_NODES) % NW
    wser = np.where(r % 2 == 0, c, NW - 1 - c)
    node_w[order] = wser
    node_pos[order] = r
    loads = np.bincount(node_w, weights=deg.astype(np.float64),
                        minlength=NW).astype(np.int64)
    for _ in range(2000):
        wmax = int(np.argmax(loads))
        wmin = int(np.argmin(loads))
        gap = loads[wmax] - loads[wmin]
        if gap <= 1:
            break
        na = np.where(node_w == wmax)[0]
        nb = np.where(node_w == wmin)[0]
        da, db = deg[na], deg[nb]
        tgt = gap / 2.0
        diff = da[:, None] - db[None, :]
        diff = np.where(diff > 0, diff, 0)
        i, j = np.unravel_index(np.argmin(np.abs(diff - tgt)), diff.shape)
        if diff[i, j] <= 0:
            break
        a, b = na[i], nb[j]
        node_w[a], node_w[b] = wmin, wmax
        node_pos[a], node_pos[b] = node_pos[b], node_pos[a]
        loads[wmax] -= diff[i, j]
        loads[wmin] += diff[i, j]
    K = int((loads.max() + P - 1) // P)
    return node_w, node_pos, K


def _preprocess(feat, W_src, b_src, W_dst, b_dst, attn_e, W_res, b_res, src, dst):
    feat = np.asarray(feat, dtype=np.float32)
    for b in (b_src, b_dst, b_res):
        assert not np.asarray(b, np.float32).any(), \
            "nonzero biases not supported by this kernel build"
    src = np.asarray(src, dtype=np.int64)
    dst = np.asarray(dst, dtype=np.int64)

    node_w, node_pos, K = _balance(dst)
    n_tiles = WIN * K
    n_groups = (n_tiles + GRP - 1) // GRP
    n_groups += n_groups % 2          # even: sede DMA fetches group pairs
    n_slots = n_groups * GRP * P

    gw = node_w[dst]
    order_e = np.argsort(gw, kind="stable")
    gw_s = gw[order_e]
    src_s = src[order_e]
    dst_s = dst[order_e]
    starts = np.searchsorted(gw_s, np.arange(M * WIN))
    idx_in_w = np.arange(len(dst)) - starts[gw_s]
    dev_e = gw_s // WIN
    tau_e = (gw_s % WIN) * K + idx_in_w // P
    slot_e = tau_e * P + idx_in_w % P

    feat16 = feat.astype(np.float16)
    feat16T = np.ascontiguousarray(feat16.T)             # [128, N]

    nodemap = np.full((M, WIN, P), -1, dtype=np.int64)
    nodemap[node_w // WIN, node_w % WIN, node_pos] = np.arange(N_NODES)

    attn_f = np.asarray(attn_e, np.float32).reshape(HF)
    asel = np.zeros((HF, N_HEADS), dtype=np.float16)
    asel[np.arange(HF), np.arange(HF) // OUT_FEATS] = \
        (0.4 * attn_f).astype(np.float16)
    # linear score part: 0.6 * sum_f attn[h,f] * W_src[hf,:]
    aW = attn_f[:, None] * np.asarray(W_src, np.float32)     # [HF, IN]
    wu = 0.6 * aW.reshape(N_HEADS, OUT_FEATS, IN_FEATS).sum(1).T  # [IN, 4]
    cst = {
        "ws16": np.ascontiguousarray(
            np.asarray(W_src, np.float32).T).astype(np.float16),
        "wd16": np.ascontiguousarray(
            np.asarray(W_dst, np.float32).T).astype(np.float16),
        "wu16": np.ascontiguousarray(wu).astype(np.float16),
        "wr16": np.ascontiguousarray(
            np.asarray(W_res, np.float32).T).astype(np.float16),
        "asel": asel,
    }

    nsplit = (n_groups + GPP - 1) // GPP
    opp = (n_groups + OSPLIT - 1) // OSPLIT
    in_maps = []
    for d in range(M):
        m = dict(cst)
        sel = dev_e == d
        sl = slot_e[sel]
        se_ids = np.zeros(n_slots, dtype=np.int64)
        de_ids = np.zeros(n_slots, dtype=np.int64)
        pos = np.full(n_slots, -1, dtype=np.int64)
        se_ids[sl] = src_s[sel]
        de_ids[sl] = dst_s[sel]
        pos[sl] = node_pos[dst_s[sel]]

        fse = feat16T[:, se_ids].reshape(P, n_groups, GRP * P)
        fde = feat16T[:, de_ids].reshape(P, n_groups, GRP * P)
        sede = np.empty((P, n_groups, 2, GRP * P), dtype=np.float16)
        sede[:, :, 0, :] = fse
        sede[:, :, 1, :] = fde
        sede = sede.reshape(P, n_groups * 2 * GRP * P)
        for i in range(nsplit):
            g0, g1 = GPP * i, min(GPP * (i + 1), n_groups)
            m[f"sede{i}"] = np.ascontiguousarray(
                sede[:, g0 * 2 * GRP * P:g1 * 2 * GRP * P])

        posmat = pos.reshape(n_groups, GRP, P)           # [g, t, p]
        oh = (posmat[:, :, :, None] ==
              np.arange(P)[None, None, None, :]).astype(np.float16)
        oh = np.ascontiguousarray(oh.transpose(2, 0, 1, 3)).reshape(P, -1)
        for i in range(OSPLIT):
            g0, g1 = opp * i, min(opp * (i + 1), n_groups)
            m[f"oh{i}"] = np.ascontiguousarray(
                oh[:, g0 * GRP * P:g1 * GRP * P])

        ids = nodemap[d].reshape(-1)
        fw = np.zeros((OUT_ROWS, IN_FEATS), dtype=np.float16)
        valid = ids >= 0
        fw[valid] = feat16[ids[valid]]
        m["fwinT"] = np.ascontiguousarray(fw.T)
        in_maps.append(m)
    return K, n_groups, in_maps, nodemap


def kernel(feat, W_src, b_src, W_dst, b_dst, attn_e, W_res, b_res, src, dst,
           _trace=False, _trace_kwargs=None):
    K, n_groups, in_maps, nodemap = _preprocess(
        feat, W_src, b_src, W_dst, b_dst, attn_e, W_res, b_res, src, dst)
    key = (K, n_groups)
    if key not in _prog_cache:
        _prog_cache[key] = _build_program(K, n_groups)
    nc = _prog_cache[key]

    kw = {}
    if _trace:
        kw = dict(trace=True, trace_kwargs=_trace_kwargs or {})
    res = run_bass_kernel_spmd(nc, in_maps, core_ids=list(range(M)), **kw)
    full = np.empty((N_NODES, HF), dtype=np.float32)
    for d in range(M):
        ids = nodemap[d].reshape(-1)
        valid = ids >= 0
        full[ids[valid]] = res.results[d]["out_d"][valid]
    kernel._last_results = res
    kernel._last_cfg = (K, n_groups)
    return full.reshape(N_NODES, N_HEADS, OUT_FEATS)
